# revision 42
# baseline (speedup 1.0000x reference)
"""Trainium2 Bass kernel for multi-head attention (b=2, n=2048, d=512, h=8).

Sharding: batch*heads over 8 cores (2 heads of one batch element per core);
host sums the 4 per-core output-projection partials per batch.

Per-core dataflow (fp16 operands, fp32 PSUM; PV in fp8 DoubleRow):
  A) x arrives as 16 column-slice DMAs (1KB lines spread across all 128
     partitions -> all SDMA engines pull concurrently), tile-0 + Wk + Wq
     first, spread over the sync/scalar/gpsimd DGE queues. K/Q projections
     in fp16; V projected then PE-transposed to keys-on-partitions and cast
     to fp8 chunk-PAIRED DoubleRow slots [P, pair, 2, 80(66 used)] with a
     fused ones column at col 64 (slot stride must be 16B-aligned for the
     dual-fp8 LdWeights ISA rules).
  B) per 512-query tile: scores fp16 (K=64 per head, both heads packed in one
     [128,1024] PSUM tile), ScalarE exp -> pt fp16. The ScalarE exp stream
     (~1.05us per chunk) is the wall; STs are paced just ahead of it and PV
     fills the PE between. One DVE op per chunk computes pm8 = fp8(pt - 1)
     (centered: fp8 quantization noise ~2.6x smaller than on fp8(exp)). PV
     runs as fp8 DoubleRow matmuls contracting TWO key chunks per
     instruction: ot += v8[c].T pm8[c] + v8[c+1].T pm8[c+1] -- half the fp16
     PV matmul count. PSUM row 64 accumulates sum(pm) via the ones columns.
  C) normalize: den = sum(pm) + N; numerator = ot + vsum, where vsum =
     colsum(V) is computed host-side (P@V = (pm+1)@V = pm@V + colsum(V));
     one fused DVE scalar_tensor_tensor computes (ot + vsum) * recip(den).
     Output projection in fp16, partials DMA'd out fp16 with the two
     partition-halves split across the sync/gpsimd/scalar DGE queues.

End-to-end rel err ~1.17e-2 (matches the numpy fp8 simulation exactly).
"""

import numpy as np

import concourse.mybir as mybir
import concourse.tile as tile
from concourse import bacc
from concourse.bass_utils import run_bass_kernel_spmd
from concourse.masks import make_identity
from contextlib import ExitStack

P = 128
N = 2048
D = 512
DH2 = 128
QT = 512
NQT = N // QT     # 4
KC = N // P       # 16
NPAIR = KC // 2   # 8
SCALE = D ** -0.5
F32 = mybir.dt.float32
F16 = mybir.dt.float16
F8 = mybir.dt.float8e4
EXP = mybir.ActivationFunctionType.Exp
DR = mybir.MatmulPerfMode.DoubleRow
ALU = mybir.AluOpType

_CACHED = {}


def build_nc():
    nc = bacc.Bacc("TRN2", target_bir_lowering=False, debug=False, num_devices=8)

    xt_d = nc.dram_tensor("xt", [P, 4, N], F16, kind="ExternalInput")
    wq_d = nc.dram_tensor("wq", [P, 4, DH2], F16, kind="ExternalInput")
    wk_d = nc.dram_tensor("wk", [P, 4, DH2], F16, kind="ExternalInput")
    wv_d = nc.dram_tensor("wv", [P, 4, DH2], F16, kind="ExternalInput")
    wo_d = nc.dram_tensor("wo", [64, 2, D], F16, kind="ExternalInput")
    vs_d = nc.dram_tensor("vs", [64, 2], F32, kind="ExternalInput")
    out_d = nc.dram_tensor("out", [N, D], F16, kind="ExternalOutput")

    with tile.TileContext(nc) as tc, ExitStack() as ctx:
        const = ctx.enter_context(tc.tile_pool(name="const", bufs=1))
        xt_pool = ctx.enter_context(tc.tile_pool(name="xt", bufs=1))
        w_pool = ctx.enter_context(tc.tile_pool(name="w", bufs=1))
        qk_pool = ctx.enter_context(tc.tile_pool(name="qk", bufs=1))
        v_pool = ctx.enter_context(tc.tile_pool(name="v", bufs=1))
        pt_pool = ctx.enter_context(tc.tile_pool(name="pt", bufs=24))
        pm_pool = ctx.enter_context(tc.tile_pool(name="pm", bufs=10))
        on_pool = ctx.enter_context(tc.tile_pool(name="on", bufs=2))
        nrm_pool = ctx.enter_context(tc.tile_pool(name="nrm", bufs=3))
        y_sb_pool = ctx.enter_context(tc.tile_pool(name="ysb", bufs=4))
        st_pool = ctx.enter_context(tc.tile_pool(name="st_ps", bufs=2, space="PSUM"))
        ot_pool = ctx.enter_context(tc.tile_pool(name="ot_ps", bufs=1, space="PSUM"))
        misc_ps = ctx.enter_context(tc.tile_pool(name="misc_ps", bufs=2, space="PSUM"))

        identity = const.tile([P, P], F16)
        make_identity(nc, identity[:])

        def fill_const(out_ap, n_free, val):
            nc.vector.tensor_scalar(
                out=out_ap, in0=identity[:, 0:n_free],
                scalar1=0.0, scalar2=val,
                op0=ALU.mult, op1=ALU.add,
            )

        # ---- input DMAs: x column-slices (1KB lines spread across all 128
        # partitions -> all SDMA engines), tile-0 + wk + wq first, spread
        # over sync/scalar/gpsimd queues. Scalar gets few issues so the exp
        # stream starts early.
        xa_sb = xt_pool.tile([P, 4, N], F16, tag="xa")
        wk_sb = w_pool.tile([P, 4, DH2], F16, tag="wk")
        wq_sb = w_pool.tile([P, 4, DH2], F16, tag="wq")
        wv_sb = w_pool.tile([P, 4, DH2], F16, tag="wv")
        wo_sb = w_pool.tile([64, 2, D], F16, tag="wo")
        vs_sb = w_pool.tile([64, 2], F32, tag="vs")

        def xt_dma(ring, tq4, c):
            qs = slice(tq4 * QT, (tq4 + 1) * QT)
            ring.dma_start(xa_sb[:, c, qs], xt_d.ap()[:, c, qs])

        nc.sync.dma_start(wk_sb[:, 0:2, :], wk_d.ap()[:, 0:2, :])
        nc.scalar.dma_start(wk_sb[:, 2:4, :], wk_d.ap()[:, 2:4, :])
        xt_dma(nc.sync, 0, 0)
        xt_dma(nc.scalar, 0, 2)
        xt_dma(nc.gpsimd, 0, 1)
        xt_dma(nc.gpsimd, 0, 3)
        nc.sync.dma_start(wq_sb[:, 0:2, :], wq_d.ap()[:, 0:2, :])
        nc.scalar.dma_start(wq_sb[:, 2:4, :], wq_d.ap()[:, 2:4, :])
        for tq4 in (1, 2, 3):
            xt_dma(nc.sync, tq4, 0)
            xt_dma(nc.sync, tq4, 1)
            xt_dma(nc.gpsimd, tq4, 2)
            xt_dma(nc.gpsimd, tq4, 3)
            if tq4 == 1:
                nc.sync.dma_start(wv_sb[:], wv_d.ap())
            if tq4 == 2:
                nc.gpsimd.dma_start(wo_sb[:], wo_d.ap())
                nc.gpsimd.dma_start(vs_sb[:], vs_d.ap())

        out_rings = [nc.sync, nc.gpsimd, nc.sync, nc.gpsimd]

        w_sbs = {0: wk_sb, 1: wq_sb, 2: wv_sb}

        qT = qk_pool.tile([P, N], F16, tag="qT")
        kT = qk_pool.tile([P, N], F16, tag="kT")
        vT = qk_pool.tile([P, N], F16, tag="vT")
        # fp8 V in chunk-paired DoubleRow slot layout; col 64 = ones (rowsum),
        # col 65 = zeros, slot stride padded to 80 (16B alignment rule)
        v8 = [v_pool.tile([P, NPAIR, 2, 80], F8, tag=f"v8_{h}", name=f"v8_{h}")
              for h in range(2)]

        def proj_tile(tgt, which, tq4):
            ps = misc_ps.tile([P, QT], F32, tag="m", name=f"ps_{which}_{tq4}")
            for c in range(4):
                nc.tensor.matmul(
                    ps[:],
                    lhsT=w_sbs[which][:, c, :],
                    rhs=xa_sb[:, c, tq4 * QT:(tq4 + 1) * QT],
                    start=(c == 0), stop=(c == 3),
                )
            nc.vector.tensor_copy(tgt[:, tq4 * QT:(tq4 + 1) * QT], ps[:])

        def v_path():
            for tq4 in range(NQT):
                proj_tile(vT, 2, tq4)
            for h in range(2):
                for pc in range(NPAIR):
                    fill_const(v8[h][:, pc, :, 64], 2, 1.0)
                    fill_const(v8[h][:, pc, :, 65], 2, 0.0)
            for c in range(KC):
                tp = misc_ps.tile([P, P], F16, tag="m", name=f"tp_{c}")
                nc.tensor.transpose(tp[:], vT[:, c * P:(c + 1) * P], identity[:])
                pc, s = divmod(c, 2)
                nc.vector.tensor_copy(v8[0][:, pc, s, 0:64], tp[:, 0:64])
                nc.vector.tensor_copy(v8[1][:, pc, s, 0:64], tp[:, 64:128])

        # ---- stage B/C ----
        pts = {}
        pms = {}
        ots = {}

        def st_chunk(t, c):
            tq = slice(t * QT, (t + 1) * QT)
            st = st_pool.tile([P, 2 * QT], F32, tag="st", name=f"st_{t}_{c}")
            for h in range(2):
                hp = 64 * h
                nc.tensor.matmul(
                    st[:, h * QT:(h + 1) * QT],
                    lhsT=kT[hp:hp + 64, c * P:(c + 1) * P],
                    rhs=qT[hp:hp + 64, tq],
                    start=True, stop=True,
                )
            pt = pt_pool.tile([P, 2 * QT], F16, tag="pt", name=f"pt_{t}_{c}")
            nc.scalar.activation(pt[:], st[:], EXP, scale=SCALE)
            pts[(t, c)] = pt

        def pm_sub(t, c, on_pool=False):
            # pm8[pair][:, slot, :] = fp8(pt - 1); one vector op per chunk.
            # In the last window the GpSimd engine is idle, so the final
            # pairs run there in parallel with the DVE stream.
            pc, s = divmod(c, 2)
            if s == 0:
                pms[(t, pc)] = pm_pool.tile([P, 2, 2 * QT], F8, tag="pm",
                                            name=f"pm_{t}_{pc}")
            pt = pts.pop((t, c))
            nc.vector.tensor_scalar_add(
                out=pms[(t, pc)][:, s, :], in0=pt[:], scalar1=-1.0,
            )

        def pv_pair(t, pc):
            if pc == 0:
                ots[t] = (ot_pool.tile([66, QT], F32, tag="ot0", name=f"ot0_{t}"),
                          ot_pool.tile([66, QT], F32, tag="ot1", name=f"ot1_{t}"))
            pm = pms.pop((t, pc))
            for h in range(2):
                rhs = pm[:, :, h * QT:(h + 1) * QT]
                nc.tensor.matmul(
                    ots[t][h][:],
                    lhsT=v8[h][:, pc, :, 0:66],
                    rhs=rhs,
                    start=(pc == 0), stop=(pc == NPAIR - 1),
                    perf_mode=DR,
                    skip_group_check=True,
                )

        on_ts = {}

        def norm_head(t, h, phase):
            ot = ots[t][h]
            if phase == 0:
                sums = nrm_pool.tile([1, QT], F32, tag=f"sums{h}",
                                     name=f"sums_{t}_{h}")
                nc.vector.tensor_scalar_add(
                    out=sums[:], in0=ot[64:65, :], scalar1=float(N),
                )
                rsum = nrm_pool.tile([1, QT], F32, tag=f"rsum{h}",
                                     name=f"rsum_{t}_{h}")
                nc.vector.reciprocal_approx_fast(rsum[:], sums[:])
                rcb = nrm_pool.tile([64, QT], F32, tag=f"rcb{h}",
                                    name=f"rcb_{t}_{h}")
                nc.gpsimd.partition_broadcast(rcb[:], rsum[:], channels=64)
                norm_head.rcbs[(t, h)] = rcb
            else:
                on_h = on_pool.tile([64, QT], F16, tag=f"on{h}",
                                    name=f"on_{t}_{h}")
                nc.vector.scalar_tensor_tensor(
                    out=on_h[:], in0=ot[0:64, :], scalar=vs_sb[:, h:h + 1],
                    in1=norm_head.rcbs.pop((t, h)), op0=ALU.add, op1=ALU.mult,
                )
                on_ts.setdefault(t, []).append(on_h)
        norm_head.rcbs = {}

        def out_proj_qc(t, qc):
            rings = [nc.sync, nc.gpsimd, nc.scalar, nc.sync]
            on_t = on_ts[t]
            yps = misc_ps.tile([P, D], F32, tag="m", name=f"y_{t}_{qc}")
            for h in range(2):
                nc.tensor.matmul(
                    yps[:],
                    lhsT=on_t[h][:, qc * P:(qc + 1) * P],
                    rhs=wo_sb[:, h, :],
                    start=(h == 0), stop=(h == 1),
                )
            ysb = y_sb_pool.tile([P, D], F16, tag="ysb", name=f"ysb_{t}_{qc}")
            nc.vector.tensor_copy(ysb[:], yps[:])
            row = (t * 4 + qc) * P
            # split by partition halves so two DMA engines write in parallel
            rings[qc].dma_start(out_d.ap()[row:row + 64, :], ysb[0:64, :])
            rings[qc ^ 1].dma_start(out_d.ap()[row + 64:row + P, :],
                                    ysb[64:128, :])

        # t=0: interleave K/Q projections with the first ST chunks so the
        # exp stream starts as early as possible
        proj_tile(kT, 0, 0)
        proj_tile(qT, 1, 0)
        for tq4 in range(NQT):
            if tq4 > 0:
                proj_tile(kT, 0, tq4)
            for c in range(tq4 * 4, tq4 * 4 + 4):
                st_chunk(0, c)
            if tq4 > 0:
                proj_tile(qT, 1, tq4)
        v_path()

        # steady windows: STs of tile t paced against pm-sub/PV of tile t-1
        for t in range(1, NQT + 1):
            pv_t = t - 1
            for c in range(KC):
                pm_sub(pv_t, c)   # DVE; all pt(t-1) ready at window start
            pv_pair(pv_t, 0)
            for i in range(KC - 2):
                if t < NQT:
                    st_chunk(t, i)
                if i % 2 == 1 and i // 2 + 1 < NPAIR:
                    pv_pair(pv_t, i // 2 + 1)
            norm_head(pv_t, 0, 0)
            norm_head(pv_t, 1, 0)
            norm_head(pv_t, 0, 1)
            norm_head(pv_t, 1, 1)
            if t < NQT:
                st_chunk(t, KC - 2)
            out_proj_qc(pv_t, 0)
            out_proj_qc(pv_t, 1)
            if t < NQT:
                st_chunk(t, KC - 1)
            out_proj_qc(pv_t, 2)
            out_proj_qc(pv_t, 3)
            del on_ts[pv_t]
            del ots[pv_t]

    nc.compile()
    return nc


def make_in_maps(x, Wq, Wk, Wv, Wo):
    """Shard full inputs into the 8 per-core input dicts (host-side fp16)."""
    in_maps = []
    xsum = x.sum(axis=1)  # [2, 512]
    for core in range(8):
        b, p = divmod(core, 4)
        r = slice(p * DH2, (p + 1) * DH2)
        # xt[p, c, n] = x[b, n, c*128 + p]
        xt = x[b].T.reshape(4, P, N).transpose(1, 0, 2)
        wq = Wq[r, :].T.reshape(4, P, DH2).transpose(1, 0, 2)
        wk = Wk[r, :].T.reshape(4, P, DH2).transpose(1, 0, 2)
        wv = Wv[r, :].T.reshape(4, P, DH2).transpose(1, 0, 2)
        wo = Wo[:, r].T.reshape(2, 64, D).transpose(1, 0, 2)
        vsum = (xsum[b] @ Wv[r, :].T).reshape(2, 64).T            # [64, 2]
        in_maps.append({
            "xt": np.ascontiguousarray(xt, dtype=np.float16),
            "wq": np.ascontiguousarray(wq, dtype=np.float16),
            "wk": np.ascontiguousarray(wk, dtype=np.float16),
            "wv": np.ascontiguousarray(wv, dtype=np.float16),
            "wo": np.ascontiguousarray(wo, dtype=np.float16),
            "vs": np.ascontiguousarray(vsum, dtype=np.float32),
        })
    return in_maps


def kernel(x, mask, Wq, Wk, Wv, Wo, bo, _trace=False):
    x = np.asarray(x, dtype=np.float32)
    Wq = np.asarray(Wq, dtype=np.float32)
    Wk = np.asarray(Wk, dtype=np.float32)
    Wv = np.asarray(Wv, dtype=np.float32)
    Wo = np.asarray(Wo, dtype=np.float32)
    bo = np.asarray(bo, dtype=np.float32)
    # mask is additive and all-zeros per the problem spec -> ignored

    if "nc" not in _CACHED:
        _CACHED["nc"] = build_nc()
    nc = _CACHED["nc"]

    in_maps = make_in_maps(x, Wq, Wk, Wv, Wo)
    res = run_bass_kernel_spmd(nc, in_maps, core_ids=list(range(8)), trace=_trace)
    parts = [res.results[c]["out"].astype(np.float32) for c in range(8)]
    out = np.empty((2, N, D), dtype=np.float32)
    for b in range(2):
        out[b] = parts[4 * b] + parts[4 * b + 1] + parts[4 * b + 2] + parts[4 * b + 3]
    out += bo[None, None, :]
    _CACHED["last_exec_time_ns"] = res.exec_time_ns
    _CACHED["res"] = res
    return out

